# revision 2
# baseline (speedup 1.0000x reference)
"""V3: bilinear pooling classifier with PSUM column-accumulation.

Math (same identities as baseline kernel.py):
    out[b,c] = g_b^T A_c g_b / norm_b + bias_c,  g = sign(f)sqrt(|f|),
    A_c = upper-tri symmetrized blocks of W[c].reshape(D,D).

Per-core split: core k owns block-columns {k, 15-k} of the 16x16 block
upper triangle -> 17 blocks/class. Stage-1 accumulates each column's
blocks in PSUM (over bi), so stage-2/3 egress is per-chain [128,32]
instead of per-block.

SPMD uniformity: chain lengths in the shared program are fixed at
(1,2,2,4,8) (subset sums cover 1..8, so every core can split its two
columns k+1 / 16-k across the 5 chains). Host packs, per core, which
block feeds op t and which g-block is the moving operand; the program
only sees fixed offsets and start/stop flags.

Schedule: 10 groups x 3 classes. Within a group the 17 ops interleave
round-robin across the 3 classes so consecutive PE matmuls target
different PSUM regions (accumulation chains don't serialize the PE).
Stage 2: DVE multiplies ps[128,480] by g_bj pattern -> v (bf16).
Stage 3: ones-matmul reduces partitions -> ps2[1,480]; ACT copies to
obuf. Final DMA once after the repeat loop.

W is SBUF-resident (130KB/partition), loaded before the repeat loop.
"""

import sys

import numpy as np

if "/opt/trn_rl_repo" not in sys.path:
    sys.path.insert(0, "/opt/trn_rl_repo")

import ml_dtypes

import concourse.bass as bass
import concourse.bacc as bacc
import concourse.mybir as mybir
import concourse.tile as tile
from concourse.bass_utils import run_bass_kernel_spmd

B, D, C = 32, 2048, 30
EPS_SQRT = 1e-10
EPS_NORM = 1e-12

N_CORES = 8
P = 128
NB = D // P            # 16
NS = 17                # blocks per core per class
NG = 10                # groups of 3 classes
GC = 3                 # classes per group
CHAINS = (1, 2, 2, 4, 8)   # fixed chain lengths, sum = 17
NCH = len(CHAINS)
GW = GC * NCH * B      # 480 group width in PSUM

# chain index ranges in op order t = 0..16
CH_START = []
CH_STOP = []
_t0 = 0
for L in CHAINS:
    CH_START.append(_t0)
    CH_STOP.append(_t0 + L - 1)
    _t0 += L

# subset of chains assigned to column k (sum of lengths == k+1)
SUBSET = {
    1: (0,), 2: (1,), 3: (0, 1), 4: (3,), 5: (0, 3), 6: (1, 3),
    7: (0, 1, 3), 8: (4,),
}

WDT = "bf16"           # "bf16" | "fp8e3"
WSCALE = 128.0         # host premultiplier for fp8 (power of 2)

_CACHE = {}

# When True, _build_bass declares W as an Internal (device-resident,
# uninitialized) DRAM tensor instead of an input. Timing-only: removes the
# 135MB/call host->device input copy so the slope signal isn't buried in
# transfer noise. Execution time is data-independent.
BENCH_INTERNAL_W = False


def _cache_key(r):
    return ("nc", WDT, BENCH_INTERNAL_W, r)


def _wdt():
    return mybir.dt.bfloat16 if WDT == "bf16" else mybir.dt.float8e3


def _build_bass(repeat=1):
    wdt = _wdt()
    nc = bacc.Bacc(None, target_bir_lowering=False, debug=False)
    wkind = "Internal" if BENCH_INTERNAL_W else "ExternalInput"
    w_d = nc.dram_tensor("w", [P, C * NS * P], wdt, kind=wkind)
    gt_d = nc.dram_tensor("gt", [P, NS * B], mybir.dt.bfloat16,
                          kind="ExternalInput")
    gc_d = nc.dram_tensor("gc", [P, GC, NCH * B], mybir.dt.float32,
                          kind="ExternalInput")
    out_d = nc.dram_tensor("out", [1, NG * GW], mybir.dt.float32,
                           kind="ExternalOutput")

    with tile.TileContext(nc) as tc:
        with (
            tc.tile_pool(name="const", bufs=1) as cpool,
            tc.tile_pool(name="spool", bufs=3) as spool,
            tc.tile_pool(name="psA", bufs=2, space=bass.MemorySpace.PSUM) as ppoolA,
            tc.tile_pool(name="psB", bufs=2, space=bass.MemorySpace.PSUM) as ppoolB,
        ):
            wt = cpool.tile([P, C * NS * P], wdt)
            # W preload: split into 4 DMAs so HWDGE pipelines them
            q = C * NS * P // 4
            for i in range(4):
                nc.sync.dma_start(wt[:, i * q:(i + 1) * q],
                                  w_d[:, i * q:(i + 1) * q])
            gt = cpool.tile([P, NS * B], mybir.dt.bfloat16)
            nc.scalar.dma_start(gt[:], gt_d[:])
            gc = cpool.tile([P, GC, NCH * B], mybir.dt.float32)
            nc.scalar.dma_start(gc[:], gc_d[:])
            ones = cpool.tile([P, 1], mybir.dt.bfloat16)
            nc.vector.memset(ones[:], 1.0)
            obuf = cpool.tile([1, NG * GW], mybir.dt.float32)

            CW = NCH * B             # 160 used cols per class
            BANK = 512               # fp32 elems per PSUM bank

            # stage-2/3 of group g is emitted in the middle of group g+1's
            # matmul stream so the PE never waits on the DVE multiply
            pending = []             # [(v, obuf_slice)]

            def flush_pending():
                v_, dst = pending.pop(0)
                ps2 = ppoolB.tile([1, GW], mybir.dt.float32)
                nc.tensor.matmul(ps2[:], ones[:], v_[:], start=True, stop=True)
                nc.scalar.copy(dst, ps2[:])

            for _ in range(repeat):
                for gi in range(NG):
                    # one bank per class: chains of a class never interleave
                    # with another class's start=True (has_written clear is
                    # bank-granular)
                    ps = ppoolA.tile([P, GC, BANK], mybir.dt.float32)
                    for t in range(NS):
                        ch = next(i for i in range(NCH)
                                  if CH_START[i] <= t <= CH_STOP[i])
                        for ci in range(GC):
                            c = gi * GC + ci
                            off = (c * NS + t) * P
                            nc.tensor.matmul(
                                ps[:, ci, ch * B:(ch + 1) * B],
                                wt[:, off:off + P],
                                gt[:, t * B:(t + 1) * B],
                                start=(t == CH_START[ch]),
                                stop=(t == CH_STOP[ch]),
                            )
                        if t == 7 and pending:
                            flush_pending()
                    v = spool.tile([P, GC, CW], mybir.dt.bfloat16)
                    nc.vector.tensor_mul(v[:], ps[:, :, :CW], gc[:])
                    pending.append((v, obuf[:, gi * GW:(gi + 1) * GW]))
            while pending:
                flush_pending()
            nc.sync.dma_start(out_d[:], obuf[:])
    if not nc.is_finalized():
        nc.finalize()
    return nc


def _core_plan(k):
    """Returns ops[t] = (bi, bj) for core k, t = 0..16, chain-major."""
    cols = (k, 15 - k)
    sub0 = SUBSET[k + 1]          # chains covering column k
    ops = [None] * NS
    bi0 = 0
    bi1 = 0
    for ch in range(NCH):
        L = CHAINS[ch]
        if ch in sub0:
            for p_ in range(L):
                ops[CH_START[ch] + p_] = (bi0, cols[0])
                bi0 += 1
        else:
            for p_ in range(L):
                ops[CH_START[ch] + p_] = (bi1, cols[1])
                bi1 += 1
    assert bi0 == cols[0] + 1 and bi1 == cols[1] + 1
    return ops


def _chain_col(k):
    """bj for each chain on core k."""
    sub0 = SUBSET[k + 1]
    return [k if ch in sub0 else 15 - k for ch in range(NCH)]


def _prep_inputs(feat, W):
    feat = np.asarray(feat, dtype=np.float32)
    W = np.asarray(W, dtype=np.float32)

    g = np.sign(feat) * np.sqrt(np.abs(feat))
    norm = np.sqrt(np.sum(np.abs(feat), axis=1, dtype=np.float64) ** 2
                   + EPS_SQRT * float(D) * float(D))
    norm = np.maximum(norm, EPS_NORM)

    W4 = W.reshape(C, NB, P, NB, P)      # [c, bi, i, bj, j]
    gT = np.ascontiguousarray(g.T)       # [D, B] fp32

    wnp_dt = ml_dtypes.bfloat16 if WDT == "bf16" else ml_dtypes.float8_e3m4
    wmul = 1.0 if WDT == "bf16" else WSCALE
    gdiv = 1.0 if WDT == "bf16" else WSCALE

    in_maps = []
    for k in range(N_CORES):
        ops = _core_plan(k)
        wk = np.empty((P, C * NS * P), dtype=wnp_dt)
        for c in range(C):
            for t, (bi, bj) in enumerate(ops):
                blk = W4[c, bi, :, bj, :]
                if bi != bj:
                    blk = blk + W4[c, bj, :, bi, :].T
                off = (c * NS + t) * P
                wk[:, off:off + P] = (blk * wmul).astype(wnp_dt)
        gt = np.empty((P, NS, B), dtype=np.float32)
        for t, (bi, bj) in enumerate(ops):
            gt[:, t, :] = gT[bi * P:(bi + 1) * P, :]
        ccol = _chain_col(k)
        gc = np.empty((P, GC, NCH, B), dtype=np.float32)
        for ch in range(NCH):
            bj = ccol[ch]
            gc[:, :, ch, :] = (gT[bj * P:(bj + 1) * P, :] / gdiv)[:, None, :]
        in_maps.append({
            "w": wk,
            "gt": gt.reshape(P, NS * B).astype(ml_dtypes.bfloat16),
            "gc": np.ascontiguousarray(gc.reshape(P, GC, NCH * B)),
        })
    return in_maps, norm


def _run(inputs, trace=False, repeat=1):
    feat, W, b = inputs["feat"], inputs["W"], inputs["b"]
    assert feat.shape == (B, D) and W.shape == (C, D * D)

    key = ("nc", WDT, repeat)
    if key not in _CACHE:
        _CACHE[key] = _build_bass(repeat)
    nc = _CACHE[key]

    in_maps, norm = _prep_inputs(feat, W)
    res = run_bass_kernel_spmd(nc, in_maps, list(range(N_CORES)), trace=trace)
    parts = np.stack([r["out"] for r in res.results]).astype(np.float64)
    # [core, group, class-in-group, chain, b] -> sum over cores and chains
    parts = parts.reshape(N_CORES, NG, GC, NCH, B).sum(axis=(0, 3))
    parts = parts.reshape(C, B).T     # [B, C]
    out = parts / norm[:, None] + np.asarray(b, dtype=np.float64)[None, :]
    return out.astype(np.float32), res


def kernel(**inputs):
    return _run(inputs)[0]


# revision 3
# speedup vs baseline: 1.0790x; 1.0790x over previous
"""V3: bilinear pooling classifier with PSUM column-accumulation.

Math (same identities as baseline kernel.py):
    out[b,c] = g_b^T A_c g_b / norm_b + bias_c,  g = sign(f)sqrt(|f|),
    A_c = upper-tri symmetrized blocks of W[c].reshape(D,D).

Per-core split: core k owns block-columns {k, 15-k} of the 16x16 block
upper triangle -> 17 blocks/class. Stage-1 accumulates each column's
blocks in PSUM (over bi), so stage-2/3 egress is per-chain [128,32]
instead of per-block.

SPMD uniformity: chain lengths in the shared program are fixed at
(1,2,2,4,8) (subset sums cover 1..8, so every core can split its two
columns k+1 / 16-k across the 5 chains). Host packs, per core, which
block feeds op t and which g-block is the moving operand; the program
only sees fixed offsets and start/stop flags.

Schedule: 10 groups x 3 classes. Within a group the 17 ops interleave
round-robin across the 3 classes so consecutive PE matmuls target
different PSUM regions (accumulation chains don't serialize the PE).
Stage 2: DVE multiplies ps[128,480] by g_bj pattern -> v (bf16).
Stage 3: ones-matmul reduces partitions -> ps2[1,480]; ACT copies to
obuf. Final DMA once after the repeat loop.

W is SBUF-resident (130KB/partition), loaded before the repeat loop.
"""

import sys

import numpy as np

if "/opt/trn_rl_repo" not in sys.path:
    sys.path.insert(0, "/opt/trn_rl_repo")

import ml_dtypes

import concourse.bass as bass
import concourse.bacc as bacc
import concourse.mybir as mybir
import concourse.tile as tile
from concourse.bass_utils import run_bass_kernel_spmd

B, D, C = 32, 2048, 30
EPS_SQRT = 1e-10
EPS_NORM = 1e-12

N_CORES = 8
P = 128
NB = D // P            # 16
NS = 17                # blocks per core per class
NG = 10                # groups of 3 classes
GC = 3                 # classes per group
CHAINS = (1, 2, 2, 4, 8)   # fixed chain lengths, sum = 17
NCH = len(CHAINS)
GW = GC * NCH * B      # 480 group width in PSUM

# chain index ranges in op order t = 0..16
CH_START = []
CH_STOP = []
_t0 = 0
for L in CHAINS:
    CH_START.append(_t0)
    CH_STOP.append(_t0 + L - 1)
    _t0 += L

# subset of chains assigned to column k (sum of lengths == k+1)
SUBSET = {
    1: (0,), 2: (1,), 3: (0, 1), 4: (3,), 5: (0, 3), 6: (1, 3),
    7: (0, 1, 3), 8: (4,),
}

WDT = "bf16"           # "bf16" | "fp8e3"
WSCALE = 128.0         # host premultiplier for fp8 (power of 2)

_CACHE = {}

# When True, _build_bass declares W as an Internal (device-resident,
# uninitialized) DRAM tensor instead of an input. Timing-only: removes the
# 135MB/call host->device input copy so the slope signal isn't buried in
# transfer noise. Execution time is data-independent.
BENCH_INTERNAL_W = False


def _cache_key(r):
    return ("nc", WDT, BENCH_INTERNAL_W, r)


def _wdt():
    return mybir.dt.bfloat16 if WDT == "bf16" else mybir.dt.float8e3


def _build_bass(repeat=1):
    wdt = _wdt()
    nc = bacc.Bacc(None, target_bir_lowering=False, debug=False)
    wkind = "Internal" if BENCH_INTERNAL_W else "ExternalInput"
    w_d = nc.dram_tensor("w", [P, C * NS * P], wdt, kind=wkind)
    gt_d = nc.dram_tensor("gt", [P, NS * B], mybir.dt.bfloat16,
                          kind="ExternalInput")
    gc_d = nc.dram_tensor("gc", [P, GC, NCH * B], mybir.dt.float32,
                          kind="ExternalInput")
    out_d = nc.dram_tensor("out", [1, NG * GW], mybir.dt.float32,
                           kind="ExternalOutput")

    with tile.TileContext(nc) as tc:
        with (
            tc.tile_pool(name="const", bufs=1) as cpool,
            tc.tile_pool(name="spool", bufs=6) as spool,
            tc.tile_pool(name="psA", bufs=2, space=bass.MemorySpace.PSUM) as ppoolA,
            tc.tile_pool(name="psB", bufs=2, space=bass.MemorySpace.PSUM) as ppoolB,
        ):
            wt = cpool.tile([P, C * NS * P], wdt)
            # W preload: split into 4 DMAs so HWDGE pipelines them
            q = C * NS * P // 4
            for i in range(4):
                nc.sync.dma_start(wt[:, i * q:(i + 1) * q],
                                  w_d[:, i * q:(i + 1) * q])
            gt = cpool.tile([P, NS * B], mybir.dt.bfloat16)
            nc.scalar.dma_start(gt[:], gt_d[:])
            gc = cpool.tile([P, GC, NCH * B], mybir.dt.float32)
            nc.scalar.dma_start(gc[:], gc_d[:])
            ones = cpool.tile([P, 1], mybir.dt.bfloat16)
            nc.vector.memset(ones[:], 1.0)
            obuf = cpool.tile([1, NG * GW], mybir.dt.float32)

            CW = NCH * B             # 160 used cols per class
            BANK = 512               # fp32 elems per PSUM bank

            # stage-2/3 of group g is emitted in the middle of group g+1's
            # matmul stream so the PE never waits on the DVE multiply
            pending = []             # [(v, obuf_slice)]

            def flush_pending():
                v_, dst = pending.pop(0)
                ps2 = ppoolB.tile([1, GW], mybir.dt.float32)
                nc.tensor.matmul(ps2[:], ones[:], v_[:], start=True, stop=True)
                nc.scalar.copy(dst, ps2[:])

            for _ in range(repeat):
                for gi in range(NG):
                    # one bank per class: chains of a class never interleave
                    # with another class's start=True (has_written clear is
                    # bank-granular)
                    ps = ppoolA.tile([P, GC, BANK], mybir.dt.float32)
                    for t in range(NS):
                        ch = next(i for i in range(NCH)
                                  if CH_START[i] <= t <= CH_STOP[i])
                        for ci in range(GC):
                            c = gi * GC + ci
                            off = (c * NS + t) * P
                            nc.tensor.matmul(
                                ps[:, ci, ch * B:(ch + 1) * B],
                                wt[:, off:off + P],
                                gt[:, t * B:(t + 1) * B],
                                start=(t == CH_START[ch]),
                                stop=(t == CH_STOP[ch]),
                            )
                        if t == 7 and pending:
                            flush_pending()
                    v = spool.tile([P, GC, CW], mybir.dt.bfloat16)
                    nc.vector.tensor_mul(v[:], ps[:, :, :CW], gc[:])
                    pending.append((v, obuf[:, gi * GW:(gi + 1) * GW]))
            while pending:
                flush_pending()
            nc.sync.dma_start(out_d[:], obuf[:])
    if not nc.is_finalized():
        nc.finalize()
    return nc


def _core_plan(k):
    """Returns ops[t] = (bi, bj) for core k, t = 0..16, chain-major."""
    cols = (k, 15 - k)
    sub0 = SUBSET[k + 1]          # chains covering column k
    ops = [None] * NS
    bi0 = 0
    bi1 = 0
    for ch in range(NCH):
        L = CHAINS[ch]
        if ch in sub0:
            for p_ in range(L):
                ops[CH_START[ch] + p_] = (bi0, cols[0])
                bi0 += 1
        else:
            for p_ in range(L):
                ops[CH_START[ch] + p_] = (bi1, cols[1])
                bi1 += 1
    assert bi0 == cols[0] + 1 and bi1 == cols[1] + 1
    return ops


def _chain_col(k):
    """bj for each chain on core k."""
    sub0 = SUBSET[k + 1]
    return [k if ch in sub0 else 15 - k for ch in range(NCH)]


def _prep_inputs(feat, W):
    feat = np.asarray(feat, dtype=np.float32)
    W = np.asarray(W, dtype=np.float32)

    g = np.sign(feat) * np.sqrt(np.abs(feat))
    norm = np.sqrt(np.sum(np.abs(feat), axis=1, dtype=np.float64) ** 2
                   + EPS_SQRT * float(D) * float(D))
    norm = np.maximum(norm, EPS_NORM)

    W4 = W.reshape(C, NB, P, NB, P)      # [c, bi, i, bj, j]
    gT = np.ascontiguousarray(g.T)       # [D, B] fp32

    wnp_dt = ml_dtypes.bfloat16 if WDT == "bf16" else ml_dtypes.float8_e3m4
    wmul = 1.0 if WDT == "bf16" else WSCALE
    gdiv = 1.0 if WDT == "bf16" else WSCALE

    in_maps = []
    for k in range(N_CORES):
        ops = _core_plan(k)
        wk = np.empty((P, C * NS * P), dtype=wnp_dt)
        for c in range(C):
            for t, (bi, bj) in enumerate(ops):
                blk = W4[c, bi, :, bj, :]
                if bi != bj:
                    blk = blk + W4[c, bj, :, bi, :].T
                off = (c * NS + t) * P
                wk[:, off:off + P] = (blk * wmul).astype(wnp_dt)
        gt = np.empty((P, NS, B), dtype=np.float32)
        for t, (bi, bj) in enumerate(ops):
            gt[:, t, :] = gT[bi * P:(bi + 1) * P, :]
        ccol = _chain_col(k)
        gc = np.empty((P, GC, NCH, B), dtype=np.float32)
        for ch in range(NCH):
            bj = ccol[ch]
            gc[:, :, ch, :] = (gT[bj * P:(bj + 1) * P, :] / gdiv)[:, None, :]
        in_maps.append({
            "w": wk,
            "gt": gt.reshape(P, NS * B).astype(ml_dtypes.bfloat16),
            "gc": np.ascontiguousarray(gc.reshape(P, GC, NCH * B)),
        })
    return in_maps, norm


def _run(inputs, trace=False, repeat=1):
    feat, W, b = inputs["feat"], inputs["W"], inputs["b"]
    assert feat.shape == (B, D) and W.shape == (C, D * D)

    key = ("nc", WDT, repeat)
    if key not in _CACHE:
        _CACHE[key] = _build_bass(repeat)
    nc = _CACHE[key]

    in_maps, norm = _prep_inputs(feat, W)
    res = run_bass_kernel_spmd(nc, in_maps, list(range(N_CORES)), trace=trace)
    parts = np.stack([r["out"] for r in res.results]).astype(np.float64)
    # [core, group, class-in-group, chain, b] -> sum over cores and chains
    parts = parts.reshape(N_CORES, NG, GC, NCH, B).sum(axis=(0, 3))
    parts = parts.reshape(C, B).T     # [B, C]
    out = parts / norm[:, None] + np.asarray(b, dtype=np.float64)[None, :]
    return out.astype(np.float32), res


def kernel(**inputs):
    return _run(inputs)[0]
